# revision 5
# baseline (speedup 1.0000x reference)
# Trainium2 Bass kernel for nn_CapsLayer_63934883168634.
#
# Math: the reference's routing softmax is over a size-1 axis, so the
# coupling coefficients are identically 1.0 and the 3-iteration routing
# loop is a fixed point.  The whole module reduces to
#     s[b, j, l] = sum_{i,k} inputs[b, i, k] * W[i, j, k, l]
#     vj         = squash(s, over l)
# i.e. one matmul [B, I*K] @ [I*K, J*L] = [64,16384]@[16384,512] plus a
# tiny per-(b, j) squash over L=16.
#
# Sharding: over the contraction axis I (the spec's mesh_spec "i").
# Each of the 8 cores computes a partial [64, 512] = x_c [64, 2048] @
# W_c [2048, 512] in fp16.  Per-core HBM traffic = 2.25 MiB.  The 8
# partials are summed and squashed on the host (gather/unshard step).
#
# Raw bass (no TileContext): manual semaphores, cleared both at block
# start (robust against dirty initial state) and after last use.  The
# inputs tile is packed into the same DRAM tensor as W so the lead
# sync-ring DMA carries it together with the first contraction chunks.

import numpy as np

B, I, K, J, L = 64, 2048, 8, 32, 16
IK = I * K               # full contraction length = 16384
N_CORES = 8
IKC = IK // N_CORES      # per-core contraction = 2048
M = B                    # matmul M (output partitions) = 64
N = J * L                # matmul N (free) = 512
P = 128                  # contraction chunk = PE partition dim
NCH = IKC // P           # 16 accumulating matmuls per core
AC = NCH * M             # a-tile columns (1024) at the front of `w`


_session = None


def _build_session():
    """Build + compile the Bass module once per process."""
    from contextlib import ExitStack

    import concourse.bacc as bacc
    import concourse.mybir as mybir

    f16 = mybir.dt.float16
    f32 = mybir.dt.float32

    nc = bacc.Bacc(
        "TRN2",
        target_bir_lowering=False,
        debug=False,
        enable_asserts=False,
        num_devices=N_CORES,
    )
    # One DRAM tensor per DMA (a contiguous slab): each transfer then
    # reads a fully contiguous block instead of 18 KiB-strided rows of
    # one big tensor, which is kinder to HBM paging during the ramp.
    o_d = nc.dram_tensor("o", [P, N], f16, kind="ExternalOutput").ap()

    # Symmetric self-contained rings: the packed tensor holds two
    # regions, one per HWDGE ring, each with the a columns for its own
    # 8 chunks followed by those chunks' W data (1.125 MiB per ring).
    # One of the two queues starts ~2us late (arbitration, direction
    # varies); symmetric rings make that cost minimal either way, and
    # each matmul is gated by a single DMA semaphore (ring FIFO makes
    # the region lead implicit).
    HALF = NCH // 2                     # 8 chunks per ring
    AH = HALF * M                       # a columns per region (512)
    REG = AH + HALF * N                 # region size in columns (4608)
    # (ring, first local chunk, #chunks) - local chunk within region
    DMAS = [("s", 0, 2), ("a", 0, 2), ("s", 2, 4), ("a", 2, 4),
            ("s", 6, 2), ("a", 6, 2)]
    # matmul order ~ expected arrival; (ring, local chunk) pairs with
    # every DMA-gate wait at an even index so column-packed matmul
    # pairs stay concurrent
    MM_ORDER = [("s", 0), ("s", 1), ("a", 0), ("a", 1),
                ("s", 2), ("s", 3), ("s", 4), ("s", 5),
                ("a", 2), ("a", 3), ("a", 4), ("a", 5),
                ("s", 6), ("s", 7), ("a", 6), ("a", 7)]

    import concourse.bass as bass_mod

    class _NoBarrierBlock(bass_mod.BassBlock):
        # Standard BassBlock exit emits drains + an all-engine barrier.
        # Every semaphore here is cleared by its sole waiter engine and
        # the runtime epilogue re-synchronizes all engines, so the
        # barrier only delays the epilogue; skip it.
        def __exit__(self, exc_type, exc_val, exc_tb):
            if exc_type is not None:
                return
            for engine, last_body in self.last_body.items():
                with self.bass.body(
                    last_body, parent=self.bass.cur_bb,
                    allow_existing_parent=True,
                ):
                    engine.br(self.end_bb)
            self.bass.switch_bb(self.end_bb)

    with ExitStack() as ctx:
        wt = ctx.enter_context(nc.sbuf_tensor([P, AC + NCH * N], f16))
        ot = ctx.enter_context(nc.sbuf_tensor([P, N], f16))
        ps = ctx.enter_context(nc.psum_tensor([P, N], f32))
        # one semaphore per DMA: with a shared ring semaphore, a
        # threshold of 16*k can be reached by a mix of increments from
        # later DMAs on fast engines while a slow engine still moves an
        # earlier DMA's data -- an intermittent data race.
        dma_sems = [
            ctx.enter_context(nc.semaphore(name=f"in{i}"))
            for i in range(len(DMAS))
        ]
        sem_mm = ctx.enter_context(nc.semaphore(name="mm"))
        sem_cp_s = ctx.enter_context(nc.semaphore(name="cp_s"))
        sem_cp_a = ctx.enter_context(nc.semaphore(name="cp_a"))
        sem_out = ctx.enter_context(nc.semaphore(name="out"))
        assert nc.cur_block is None
        block = _NoBarrierBlock(nc, f"block_{nc.next_id()}")
        nc.cur_block = block
        ctx.enter_context(block)
        ctx.callback(lambda: setattr(nc, "cur_block", None))

        # (ring, local chunk) -> index of the DMA that carries it
        chunk_gate = {}
        for i, (ring, c0, ng) in enumerate(DMAS):
            for c in range(c0, c0 + ng):
                chunk_gate[(ring, c)] = i

        def reg0(ring):
            return 0 if ring == "s" else REG

        def dma_cols(ring, c0, ng):
            # each region's lead DMA (c0 == 0) also carries its a half
            lo = reg0(ring) + (0 if c0 == 0 else AH + c0 * N)
            return slice(lo, reg0(ring) + AH + (c0 + ng) * N)

        w_slabs = []
        for i, (ring, c0, ng) in enumerate(DMAS):
            sl = dma_cols(ring, c0, ng)
            w_slabs.append(
                nc.dram_tensor(
                    f"w{i}", [P, sl.stop - sl.start], f16,
                    kind="ExternalInput").ap())

        @block.sync
        def _(sync):
            # clears first: safe against dirty initial state (this
            # engine issues every in/out inc that follows, and cp_s's
            # inc is >10us away behind the matmul chain)
            # issue first, clear after: the clears finish well inside
            # the >=1.5us before any DMA completion increment arrives
            for i, (ring, c0, ng) in enumerate(DMAS):
                if ring == "s":
                    sl = dma_cols(ring, c0, ng)
                    sync.dma_start(out=wt[:, sl], in_=w_slabs[i][:, :]).then_inc(
                        dma_sems[i], 16)
            for i, (ring, _, _) in enumerate(DMAS):
                if ring == "s":
                    sync.sem_clear(dma_sems[i])
            sync.sem_clear(sem_cp_s)
            sync.sem_clear(sem_out)
            # output, lower partition half.  No completion wait: the
            # fixed runtime epilogue (several us of barriers) runs after
            # this block, far longer than the 32 KiB transfer needs.
            sync.wait_ge(sem_cp_s, 1)
            # relay: cp_a must only fire after the cast completed, which
            # cp_s>=1 (a then_inc @complete) guarantees
            sync.sem_inc(sem_cp_a, 1)
            sync.dma_start(out=o_d[:M, :], in_=ot[:M, :]).then_inc(sem_out, 16)
            sync.sem_clear(sem_cp_s)

        @block.scalar
        def _(scalar):
            for i, (ring, c0, ng) in enumerate(DMAS):
                if ring == "a":
                    sl = dma_cols(ring, c0, ng)
                    scalar.dma_start(out=wt[:, sl], in_=w_slabs[i][:, :]).then_inc(
                        dma_sems[i], 16)
            for i, (ring, _, _) in enumerate(DMAS):
                if ring == "a":
                    scalar.sem_clear(dma_sems[i])
            scalar.sem_clear(sem_cp_a)
            # output, upper partition half
            scalar.wait_ge(sem_cp_a, 1)
            scalar.dma_start(out=o_d[M:, :], in_=ot[M:, :]).then_inc(sem_out, 16)
            scalar.sem_clear(sem_cp_a)

        @block.tensor
        def _(tensor):
            # each chunk's lhsT rides its region's lead DMA, which its
            # own gate subsumes via ring FIFO order
            waited = set()
            for i, (ring, c) in enumerate(MM_ORDER):
                di = chunk_gate[(ring, c)]
                if di not in waited:
                    tensor.wait_ge(dma_sems[di], 16)
                    waited.add(di)
                half = i % 2
                r0 = reg0(ring)
                mm = tensor.matmul(
                    ps[half * M:(half + 1) * M, :],
                    wt[:, r0 + c * M:r0 + (c + 1) * M],
                    wt[:, r0 + AH + c * N:r0 + AH + (c + 1) * N],
                    start=(i < 2),
                    stop=(i >= NCH - 2),
                    tile_position=(0, half * M),
                )
            mm.then_inc(sem_mm, 1)
            for sm in dma_sems:
                tensor.sem_clear(sm)

        @block.vector
        def _(vector):
            vector.sem_clear(sem_mm)
            vector.wait_ge(sem_mm, 1)
            vector.tensor_copy(ot[:, :], ps[:, :]).then_inc(sem_cp_s, 1)
            vector.sem_clear(sem_mm)

    nc.compile()
    return nc


def _swizzle(mat):
    """[IKC, F] -> [128, NCH*F] where col block c = rows [c*128,(c+1)*128)."""
    f = mat.shape[1]
    return np.ascontiguousarray(
        mat.reshape(NCH, P, f).transpose(1, 0, 2).reshape(P, NCH * f)
    )


def _make_in_maps(inputs):
    x = np.asarray(inputs["inputs"], dtype=np.float32)
    W = np.asarray(inputs["W"], dtype=np.float32)

    xf = x.reshape(B, IK).T.astype(np.float16)          # [IK, B]
    Wf = W.transpose(0, 2, 1, 3).reshape(IK, N).astype(np.float16)
    in_maps = []
    ah = NCH // 2 * B                  # 512 a-cols per region
    wh = NCH // 2 * N                  # 4096 W-cols per region
    for c in range(N_CORES):
        sl = slice(c * IKC, (c + 1) * IKC)
        a_sw = _swizzle(xf[sl])        # [128, 1024]; chunk k at cols k*B
        w_sw = _swizzle(Wf[sl])        # [128, 8192]; chunk k at cols k*N
        packed = np.concatenate([
            a_sw[:, :ah], w_sw[:, :wh],      # sync region: chunks 0-7
            a_sw[:, ah:], w_sw[:, wh:],      # scalar region: chunks 8-15
        ], axis=1)
        # one contiguous slab per DMA, mirroring the device-side plan:
        # (ring, c0, ng) -> region cols [c0==0 ? 0 : AH+c0*N, AH+(c0+ng)*N)
        reg = ah + wh
        slabs = {}
        for i, (ring, c0, ng) in enumerate(
                [("s", 0, 2), ("a", 0, 2), ("s", 2, 4), ("a", 2, 4),
                 ("s", 6, 2), ("a", 6, 2)]):
            r0 = 0 if ring == "s" else reg
            lo = r0 + (0 if c0 == 0 else ah + c0 * (N // 1))
            hi = r0 + ah + (c0 + ng) * N
            slabs[f"w{i}"] = np.ascontiguousarray(packed[:, lo:hi])
        in_maps.append(slabs)
    return in_maps


def kernel(**inputs):
    global _session
    from concourse.bass_utils import run_bass_kernel_spmd

    if _session is None:
        _session = _build_session()

    in_maps = _make_in_maps(inputs)
    try:
        res = run_bass_kernel_spmd(_session, in_maps, list(range(N_CORES)))
    except Exception:
        # the shared device occasionally reports a transient
        # NRT_EXEC_UNIT_UNRECOVERABLE; one retry clears it
        res = run_bass_kernel_spmd(_session, in_maps, list(range(N_CORES)))

    # gather/unshard: the contraction is split across cores (and across
    # the two PSUM column-tile halves), so the full s is the sum of all
    # partials; then squash over L.
    s = np.zeros((M, N), dtype=np.float32)
    for c in range(N_CORES):
        o = res.results[c]["o"].astype(np.float32)
        s += o[:M] + o[M:]
    s3 = s.reshape(B, J, L)
    s2 = np.sum(np.square(s3), axis=-1, keepdims=True)
    vj = (s2 / (1.0 + s2)) * (s3 / np.sqrt(s2 + 1e-7))
    return np.ascontiguousarray(vj.reshape(B, 1, J, L, 1).astype(np.float32))
